# revision 40
# baseline (speedup 1.0000x reference)
"""MoE routing kernel for Trainium2 (8 NeuronCores, SPMD data-parallel).

Computes, for x [4, 4096, 4096] f32, proto_k [64, 4096] f32, gate [64] f32:
    logits = relu(x @ proto_k.T / sqrt(4096) - gate)        # [B, S, 64]
    routing_weights, selected_experts = top_k(logits, k=8)  # [B, S, 8] each

Sharding: tokens (B*S = 16384) are split evenly across 8 cores (2048 each).
proto_k / gate are replicated. No collectives needed.

Numerics: the matmul runs as a 4-term fp16 hi/lo split (x = xh + 2^-11 xl,
proto = ph + 2^-11 pl, residuals pre-scaled by 2^11 on the host so they stay
in fp16's normal range). logits = xh@ph + 2^-11 (xh@pl + xl@ph) + 2^-22
xl@pl, recombined on the DVE.  Bit-noise-level agreement with the fp32
reference (max logit perturbation ~5e-8, zero top-8 index flips) while
streaming the PE at fp16 rate.

The kernel is DMA-bandwidth bound (32 MB of x per core; the two HWDGE rings
together sustain ~400 GB/s against the SBUF-fabric ceiling).  Device
program, tuned from perfetto/NTFF traces:
  - x is laid out on the host as per-pass [chunk][row 128][hi|lo][token]
    fp16 blocks so every DMA reads sequential DRAM; ~1.5 MB (2-chunk)
    transfers alternate between the two HWDGE rings (sync / scalar), which
    carry NOTHING but x triggers mid-run — any compute op on those queues
    would stall trigger issue (strict FIFO) and starve the stream.
  - Tokens are processed in UNEVEN passes of [768, 768, 384, 128] with
    512/256-token PSUM accumulation groups (6 acc banks + 2 transpose
    banks).  Each pass's epilogue is emitted a few chunks INTO the next
    pass's matmul stream, so its PE transposes never gate the next pass's
    in-order matmul queue; only the 128-token final pass's epilogue is
    exposed after the last HBM byte.
  - The epilogue runs on DVE (PSUM staging, hi/lo recombines, scale+gate
    via per-partition tensor_scalar, Max8/MaxIndex8) plus PE transposes.
    Max8 runs pre-relu (relu is monotone, so the top-8 order is identical)
    and the relu lands as a clamp on the [*, 8] outputs.
  - The `a` PSUM->SBUF copies of all groups are hoisted to the front of
    each epilogue: they free the banks the next pass's b-matmuls reuse.
    Otherwise the PE idles >3.4us at pass boundaries and the hardware
    activity monitor downclocks it to 1.2 GHz (measured: this costs ~10us).
  - Weights/gate ride the gpsimd SWDGE ring (a third x ring measured
    SLOWER: Q7 emission + descriptor-ring SBUF traffic); outputs are
    flushed once at the end on the by-then-idle HWDGE rings.
"""

import numpy as np

HIDDEN = 4096
NUM_EXPERTS = 64
TOP_K = 8
N_CORES = 8
TOKENS = 4 * 4096
T_CORE = TOKENS // N_CORES          # 2048 tokens per core
N_CHUNK = HIDDEN // 128             # 32 contraction chunks
N_SUB = T_CORE // 128               # 16 output sub-tiles of 128 tokens
LO_SCALE = np.float32(2.0 ** 11)
LO_UNSCALE = 2.0 ** -11

# uneven pass sizes: the last pass is tiny so the only epilogue that cannot
# overlap the DMA stream is as short as possible
PASS_TOK = [768, 768, 384, 128]
PASS_GROUPS = [[512, 256], [512, 256], [384], [128]]
# chunks per x dma_start per pass (transfer sizes ~1.5/1.5/0.75/0.5 MB:
# small enough that per-transfer latency (size / one ring's share of the
# fabric) stays a few microseconds, large enough to amortize overheads)
PASS_CHDMA = [2, 2, 2, 4]
assert sum(PASS_TOK) == T_CORE

_PROGRAM = None


def _split_multi_waits(nc):
    """walrus in this container rejects instructions carrying more sync waits
    than their ISA struct holds (setupSyncWait: 'Too many sync wait
    commands'); Drain takes one, S3_LW (matmul weight-load) ~two.  Normalize
    every instruction to a single wait by hoisting extras onto same-engine
    NOPs inserted immediately before the owner."""
    import bass_rust

    inserts = {}  # owner inst name -> list of wait-nop instructions
    for f in nc.m.functions:
        for bb in f.blocks:
            for inst in bb.instructions:
                si = inst.sync_info
                if si is None or len(si.on_wait) <= 1:
                    continue
                conds = list(si.on_wait)
                si.on_wait = conds[:1]
                eng = nc.engines[inst.engine]
                new_insts = []
                for w in conds[1:]:
                    nop = eng.nop(hint="split_wait")
                    nop.ins.sync_info = bass_rust.SyncInfo(
                        on_wait=[w], on_update=[]
                    )
                    new_insts.append(nop.ins)
                inserts[inst.name] = new_insts
    if not inserts:
        return
    # nop() appended the new instructions to whatever bb was current; strip
    # them from everywhere, then re-insert each right before its owner so
    # the engine observes every wait before executing the instruction.
    appended = {ni.name for nis in inserts.values() for ni in nis}
    for f in nc.m.functions:
        for bb in f.blocks:
            rebuilt = []
            changed = False
            for inst in bb.instructions:
                if inst.name in appended:
                    changed = True
                    continue
                if inst.name in inserts:
                    rebuilt.extend(inserts[inst.name])
                    changed = True
                rebuilt.append(inst)
            if changed:
                bb.instructions = rebuilt


def _build_program():
    import concourse.bass as bass
    import concourse.mybir as mybir
    import concourse.tile as tile

    f32 = mybir.dt.float32
    f16 = mybir.dt.float16
    u32 = mybir.dt.uint32
    E = NUM_EXPERTS
    N_PASS = len(PASS_TOK)

    nc = bass.Bass("TRN2", target_bir_lowering=False, debug=False)

    # per-pass x blocks, host-reordered to [chunk][row in chunk][hi|lo][tok]
    # fp16: fully sequential DRAM, contiguous per-partition lines.
    xp_d = [
        nc.dram_tensor(f"xp{p}", [N_CHUNK, 128, 2, PASS_TOK[p]], f16,
                       kind="ExternalInput")
        for p in range(N_PASS)
    ]
    # proto hi|lo, host-reordered to [row in chunk][chunk][2E]
    phpl_d = nc.dram_tensor("phpl", [128, N_CHUNK, 2 * E], f16,
                            kind="ExternalInput")
    gate_neg = nc.dram_tensor("gate_neg", [E, 1], f32, kind="ExternalInput")
    w_out = nc.dram_tensor("w_out", [128, N_SUB * TOP_K], f32, kind="ExternalOutput")
    i_out = nc.dram_tensor("i_out", [128, N_SUB * TOP_K], u32, kind="ExternalOutput")

    ident_dram = nc.inline_tensor(np.eye(E, dtype=np.float32), name="ident64")

    with tile.TileContext(nc) as tc:
        with (
            tc.tile_pool(name="const", bufs=1) as const_pool,
            tc.tile_pool(name="xa", bufs=12) as x_pool,
            tc.tile_pool(name="acc", bufs=6, space="PSUM") as acc_pool,
            tc.tile_pool(name="tp", bufs=2, space="PSUM") as tp_pool,
            tc.tile_pool(name="lg", bufs=3) as lg_pool,
            tc.tile_pool(name="outp", bufs=1) as out_pool,
        ):
            # --- constants (gpsimd SWDGE ring, ahead of its x share) ---
            phpl_sb = const_pool.tile([128, N_CHUNK, 2 * E], f16)
            # chunk 0 separately so the first matmul's weights land early
            nc.gpsimd.dma_start(phpl_sb[:, 0, :], phpl_d[:, 0, :])
            nc.gpsimd.dma_start(phpl_sb[:, 1:, :], phpl_d[:, 1:, :])
            gate_sb = const_pool.tile([E, 1], f32)
            nc.gpsimd.dma_start(gate_sb[:], gate_neg[:])
            ident_sb = const_pool.tile([E, E], f32)
            nc.gpsimd.dma_start(ident_sb[:], ident_dram[:])

            vals_sb = out_pool.tile([128, N_SUB * TOP_K], f32)
            idx_sb = out_pool.tile([128, N_SUB * TOP_K], u32)

            # x streams on the two HWDGE rings.  With compute running, the
            # SBUF write ports shared with the PE's ~600 GB/s of operand
            # reads cap DMA near ~400 GB/s, which two rings already reach;
            # adding the SWDGE ring as a third x path measured SLOWER
            # (Q7 emission + descriptor-ring SBUF traffic).
            rings = [nc.sync, nc.scalar]
            ring_state = [0]

            def next_ring():
                r = rings[ring_state[0] % 2]
                ring_state[0] += 1
                return r

            def emit_epilogue(p, groups, goff, a_accs, b_accs, base):
                # tk = [a_hi + 2^-11 (a_lo + b_hi) + 2^-22 b_lo]/64 - gate,
                # per 128-token subtile, transposed to [token, expert].
                # ENTIRELY on the DVE (copies, recombines, scale+gate via
                # per-partition tensor_scalar, 32x32-block transpose,
                # Max8/MaxIndex8): the in-order PE queue stays pure
                # accumulation matmuls and the sync/scalar queues stay pure
                # DMA triggers, so the epilogue can never stall the x
                # stream.  Max8 runs pre-relu (relu is monotone, so top-8
                # order is unchanged); the relu clamp lands on the [*, 8]
                # outputs at pass end.
                gorder = (reversed(range(len(groups))) if p == N_PASS - 1
                          else range(len(groups)))
                gorder = list(gorder)
                # all `a` PSUM->SBUF copies first: they free the PSUM banks
                # the NEXT pass's b-matmuls are waiting to reuse.  If they
                # queued behind a group's full DVE chain, the PE would stall
                # >3.4us at the pass boundary and the hardware activity
                # monitor would downclock it to 1.2 GHz.
                a_sbs = {}
                for g in gorder:
                    a_sb = lg_pool.tile(
                        [128, groups[g]], f32, name="a_sb", tag="a", bufs=2
                    )
                    nc.vector.tensor_copy(a_sb[:], a_accs[g][:])
                    a_sbs[g] = a_sb
                for g in gorder:
                    W = groups[g]
                    nsub = W // 128
                    a_sb = a_sbs[g]
                    u = lg_pool.tile([E, W], f32, name="u", tag="u")
                    nc.vector.scalar_tensor_tensor(
                        u[:], b_accs[g][0:E, :], 1.0, a_sb[E:2 * E, :],
                        bass.mybir.AluOpType.mult, bass.mybir.AluOpType.add,
                    )
                    v = lg_pool.tile([E, W], f32, name="v", tag="v")
                    nc.vector.scalar_tensor_tensor(
                        v[:], b_accs[g][E:2 * E, :], LO_UNSCALE, u[:],
                        bass.mybir.AluOpType.mult, bass.mybir.AluOpType.add,
                    )
                    comb = lg_pool.tile([E, W], f32, name="comb", tag="c")
                    nc.vector.scalar_tensor_tensor(
                        comb[:], v[:], LO_UNSCALE, a_sb[0:E, :],
                        bass.mybir.AluOpType.mult, bass.mybir.AluOpType.add,
                    )
                    sg = lg_pool.tile([E, W], f32, name="sg", tag="s")
                    nc.vector.tensor_scalar(
                        sg[:], comb[:], 1.0 / 64.0, gate_sb[:],
                        bass.mybir.AluOpType.mult, bass.mybir.AluOpType.add,
                    )
                    tk_psum = tp_pool.tile([128, 4 * E], f32, name="tk_psum")
                    for jj in range(nsub):
                        nc.tensor.transpose(
                            tk_psum[:, jj * E:(jj + 1) * E],
                            sg[:, jj * 128:(jj + 1) * 128],
                            ident_sb[:],
                        )
                    tk_sb = lg_pool.tile([128, 4 * E], f32, name="tk_sb", tag="t")
                    nc.vector.tensor_copy(
                        tk_sb[:, 0:nsub * E], tk_psum[:, 0:nsub * E]
                    )
                    gsub = base + goff[g] // 128
                    for jj in range(nsub):
                        s = gsub + jj
                        nc.vector.max(
                            vals_sb[:, s * TOP_K:(s + 1) * TOP_K],
                            tk_sb[:, jj * E:(jj + 1) * E],
                        )
                        nc.vector.max_index(
                            idx_sb[:, s * TOP_K:(s + 1) * TOP_K],
                            vals_sb[:, s * TOP_K:(s + 1) * TOP_K],
                            tk_sb[:, jj * E:(jj + 1) * E],
                        )
                # relu: clamp this pass's top-8 values (order-preserving)
                nsub_p = sum(groups) // 128
                os_ = slice(base * TOP_K, (base + nsub_p) * TOP_K)
                nc.vector.tensor_scalar_max(
                    vals_sb[:, os_], vals_sb[:, os_], 0.0
                )
                if p == N_PASS - 1:
                    # single output flush at the very end: every DMA ring
                    # carries x-stream triggers mid-run, and a flush trigger
                    # waiting on the epilogue would stall them (in-order
                    # queues).  By now both HWDGE rings are idle.
                    nc.sync.dma_start(w_out[:], vals_sb[:])
                    nc.scalar.dma_start(i_out[:], idx_sb[:])

            sub_base = 0  # running 128-token output subtile index
            pending = None  # deferred epilogue of the previous pass: its PE
            # matmuls must be emitted AFTER the next pass's accumulation
            # matmuls, or the in-order PE queue would stall the next pass's
            # compute (and therefore the x stream) on the epilogue's DVE
            # dependency chain.
            for p in range(N_PASS):
                T = PASS_TOK[p]
                groups = PASS_GROUPS[p]
                goff = np.cumsum([0] + groups)[:-1]
                a_accs = [
                    acc_pool.tile([128, w], f32, name=f"a_p{p}g{g}", tag="acc")
                    for g, w in enumerate(groups)
                ]
                b_accs = [
                    acc_pool.tile([128, w], f32, name=f"b_p{p}g{g}", tag="acc")
                    for g, w in enumerate(groups)
                ]

                # ---- x stream for this pass ----
                c = 0
                slot_of = {}
                last = N_PASS - 1
                while c < N_CHUNK:
                    if p == 0 and c == 0:
                        # split the very first chunk by stream and half so
                        # the first matmul waits on ~128 KB, not 1.5 MB
                        x_t = x_pool.tile([128, 1, 2, T], f16, name="x_h", tag="xt")
                        src = xp_d[p][0:1].rearrange("c r s t -> r c s t")
                        h = groups[0]
                        nc.sync.dma_start(x_t[:, 0, 0, 0:h], src[:, 0, 0, 0:h])
                        nc.scalar.dma_start(x_t[:, 0, 1, 0:h], src[:, 0, 1, 0:h])
                        nc.sync.dma_start(x_t[:, 0, 0, h:], src[:, 0, 0, h:])
                        nc.scalar.dma_start(x_t[:, 0, 1, h:], src[:, 0, 1, h:])
                        slot_of[0] = (x_t, 0)
                        c += 1
                    elif p == last and c == N_CHUNK - 2:
                        # final two chunks: one small transfer per HWDGE ring
                        # (lowest latency) so the last bytes land soon
                        x_t = x_pool.tile([128, 2, 2, T], f16, name="x_z", tag="xt")
                        src = xp_d[p][c:c + 2].rearrange("c r s t -> r c s t")
                        nc.sync.dma_start(x_t[:, 0], src[:, 0])
                        nc.scalar.dma_start(x_t[:, 1], src[:, 1])
                        slot_of[c] = (x_t, 0)
                        slot_of[c + 1] = (x_t, 1)
                        c += 2
                    else:
                        n = min(PASS_CHDMA[p], N_CHUNK - c)
                        if p == 0 and c <= 4:
                            n = 1  # small first transfers: fast PE ramp
                        if p == last:
                            n = min(n, N_CHUNK - 2 - c)
                        x_t = x_pool.tile([128, n, 2, T], f16, name="x_t", tag="xt")
                        src = xp_d[p][c:c + n].rearrange("c r s t -> r c s t")
                        next_ring().dma_start(x_t[:], src)
                        for j in range(n):
                            slot_of[c + j] = (x_t, j)
                        c += n

                # ---- accumulation matmuls ----
                # The previous pass's epilogue (which contains PE matmuls
                # gated on its DVE recombines) is emitted a few chunks INTO
                # this pass's matmul stream: late enough that its DVE inputs
                # are ready when the in-order PE reaches it, early enough
                # that its downstream DVE work isn't stuck behind this whole
                # pass (the PE absorbs the extra matmuls with its slack).
                for c in range(N_CHUNK):
                    x_t, j = slot_of[c]
                    first, lastc = (c == 0), (c == N_CHUNK - 1)
                    grange = (
                        reversed(range(len(groups)))
                        if (lastc and p == N_PASS - 1)
                        else range(len(groups))
                    )
                    for g in grange:
                        ts = slice(goff[g], goff[g] + groups[g])
                        nc.tensor.matmul(
                            a_accs[g][:], phpl_sb[:, c, :], x_t[:, j, 0, ts],
                            start=first, stop=lastc,
                        )
                        nc.tensor.matmul(
                            b_accs[g][:], phpl_sb[:, c, :], x_t[:, j, 1, ts],
                            start=first, stop=lastc,
                        )
                    if c == 6 and pending is not None:
                        emit_epilogue(*pending)
                        pending = None

                pending = (p, groups, goff, a_accs, b_accs, sub_base)
                sub_base += T // 128
            emit_epilogue(*pending)

    _split_multi_waits(nc)
    return nc


def _get_program():
    global _PROGRAM
    if _PROGRAM is None:
        _PROGRAM = _build_program()
    return _PROGRAM


def _make_in_maps(x, proto_k, gate):
    xf = np.ascontiguousarray(x, dtype=np.float32).reshape(TOKENS, HIDDEN)
    proto = np.asarray(proto_k, dtype=np.float32)
    ph = proto.astype(np.float16)
    pl = ((proto - ph.astype(np.float32)) * LO_SCALE).astype(np.float16)
    phpl = np.concatenate([ph.T, pl.T], axis=1)           # [4096, 128] f16
    # [row in chunk][chunk][2E]
    phpl_r = np.ascontiguousarray(
        phpl.reshape(N_CHUNK, 128, 2 * NUM_EXPERTS).transpose(1, 0, 2)
    )
    gate_neg = np.ascontiguousarray(
        -np.asarray(gate, dtype=np.float32).reshape(NUM_EXPERTS, 1)
    )
    toff = np.cumsum([0] + PASS_TOK)
    in_maps = []
    for cid in range(N_CORES):
        shard = xf[cid * T_CORE:(cid + 1) * T_CORE]       # [2048, 4096]
        hi = shard.astype(np.float16)
        lo = ((shard - hi.astype(np.float32)) * LO_SCALE).astype(np.float16)
        hi_t = hi.T.reshape(N_CHUNK, 128, T_CORE)         # [chunk, row, tok]
        lo_t = lo.T.reshape(N_CHUNK, 128, T_CORE)
        m = {"phpl": phpl_r, "gate_neg": gate_neg}
        for p, T in enumerate(PASS_TOK):
            xp = np.empty((N_CHUNK, 128, 2, T), np.float16)
            xp[:, :, 0, :] = hi_t[:, :, toff[p]:toff[p + 1]]
            xp[:, :, 1, :] = lo_t[:, :, toff[p]:toff[p + 1]]
            m[f"xp{p}"] = xp
        in_maps.append(m)
    return in_maps


def _gather(results):
    w = np.empty((TOKENS, TOP_K), np.float32)
    idx = np.empty((TOKENS, TOP_K), np.int32)
    for c in range(N_CORES):
        wo = results[c]["w_out"]                          # [128, 16*8]
        io = results[c]["i_out"].view(np.int32)
        w[c * T_CORE:(c + 1) * T_CORE] = (
            wo.reshape(128, N_SUB, TOP_K).transpose(1, 0, 2).reshape(T_CORE, TOP_K)
        )
        idx[c * T_CORE:(c + 1) * T_CORE] = (
            io.reshape(128, N_SUB, TOP_K).transpose(1, 0, 2).reshape(T_CORE, TOP_K)
        )
    return w.reshape(4, 4096, TOP_K), idx.reshape(4, 4096, TOP_K)


def run_sharded(in_maps, trace=False, trace_cores=None):
    from concourse.bass_utils import run_bass_kernel_spmd

    nc = _get_program()
    return run_bass_kernel_spmd(
        nc,
        in_maps,
        core_ids=list(range(N_CORES)),
        trace=trace,
        trace_cores=trace_cores,
    )


def kernel(x, proto_k, gate):
    in_maps = _make_in_maps(x, proto_k, gate)
    res = run_sharded(in_maps, trace=False)
    return _gather(res.results)


# revision 45
# speedup vs baseline: 1.0872x; 1.0872x over previous
"""MoE routing kernel for Trainium2 (8 NeuronCores, SPMD data-parallel).

Computes, for x [4, 4096, 4096] f32, proto_k [64, 4096] f32, gate [64] f32:
    logits = relu(x @ proto_k.T / sqrt(4096) - gate)        # [B, S, 64]
    routing_weights, selected_experts = top_k(logits, k=8)  # [B, S, 8] each

Sharding: tokens (B*S = 16384) are split evenly across 8 cores (2048 each).
proto_k / gate are replicated. No collectives needed.

Numerics: the matmul runs as a 4-term fp16 hi/lo split (x = xh + 2^-11 xl,
proto = ph + 2^-11 pl, residuals pre-scaled by 2^11 on the host so they stay
in fp16's normal range). logits = xh@ph + 2^-11 (xh@pl + xl@ph) + 2^-22
xl@pl, recombined on the DVE.  Bit-noise-level agreement with the fp32
reference (max logit perturbation ~5e-8, zero top-8 index flips) while
streaming the PE at fp16 rate.

The kernel is DMA-bandwidth bound (32 MB of x per core; the two HWDGE rings
together sustain ~400 GB/s against the SBUF-fabric ceiling).  Device
program, tuned from perfetto/NTFF traces:
  - x is laid out on the host as per-pass [chunk][row 128][hi|lo][token]
    fp16 blocks so every DMA reads sequential DRAM; ~1.5 MB (2-chunk)
    transfers alternate between the two HWDGE rings (sync / scalar), which
    carry NOTHING but x triggers mid-run — any compute op on those queues
    would stall trigger issue (strict FIFO) and starve the stream.
  - Tokens are processed in UNEVEN passes of [768, 768, 384, 128] with
    512/256-token PSUM accumulation groups (6 acc banks + 2 transpose
    banks).  Each pass's epilogue is emitted a few chunks INTO the next
    pass's matmul stream, so its PE transposes never gate the next pass's
    in-order matmul queue; only the 128-token final pass's epilogue is
    exposed after the last HBM byte.
  - The epilogue runs on DVE (PSUM staging, hi/lo recombines, scale+gate
    via per-partition tensor_scalar, Max8/MaxIndex8) plus PE transposes.
    Max8 runs pre-relu (relu is monotone, so the top-8 order is identical)
    and the relu lands as a clamp on the [*, 8] outputs.
  - The `a` PSUM->SBUF copies of all groups are hoisted to the front of
    each epilogue: they free the banks the next pass's b-matmuls reuse.
    Otherwise the PE idles >3.4us at pass boundaries and the hardware
    activity monitor downclocks it to 1.2 GHz (measured: this costs ~10us).
  - Weights/gate ride the gpsimd SWDGE ring (a third x ring measured
    SLOWER: Q7 emission + descriptor-ring SBUF traffic); outputs are
    flushed once at the end on the by-then-idle HWDGE rings.
"""

import numpy as np

HIDDEN = 4096
NUM_EXPERTS = 64
TOP_K = 8
N_CORES = 8
TOKENS = 4 * 4096
T_CORE = TOKENS // N_CORES          # 2048 tokens per core
N_CHUNK = HIDDEN // 128             # 32 contraction chunks
N_SUB = T_CORE // 128               # 16 output sub-tiles of 128 tokens
LO_SCALE = np.float32(2.0 ** 11)
LO_UNSCALE = 2.0 ** -11

# uneven pass sizes: the last pass is tiny so the only epilogue that cannot
# overlap the DMA stream is as short as possible
PASS_TOK = [768, 768, 384, 128]
PASS_GROUPS = [[512, 256], [512, 256], [384], [128]]
# chunks per x dma_start per pass (transfer sizes ~1.5/1.5/0.75/0.5 MB:
# small enough that per-transfer latency (size / one ring's share of the
# fabric) stays a few microseconds, large enough to amortize overheads)
PASS_CHDMA = [2, 2, 2, 4]
assert sum(PASS_TOK) == T_CORE

_PROGRAM = None


def _split_multi_waits(nc):
    """walrus in this container rejects instructions carrying more sync waits
    than their ISA struct holds (setupSyncWait: 'Too many sync wait
    commands'); Drain takes one, S3_LW (matmul weight-load) ~two.  Normalize
    every instruction to a single wait by hoisting extras onto same-engine
    NOPs inserted immediately before the owner."""
    import bass_rust

    inserts = {}  # owner inst name -> list of wait-nop instructions
    for f in nc.m.functions:
        for bb in f.blocks:
            for inst in bb.instructions:
                si = inst.sync_info
                if si is None or len(si.on_wait) <= 1:
                    continue
                conds = list(si.on_wait)
                si.on_wait = conds[:1]
                eng = nc.engines[inst.engine]
                new_insts = []
                for w in conds[1:]:
                    nop = eng.nop(hint="split_wait")
                    nop.ins.sync_info = bass_rust.SyncInfo(
                        on_wait=[w], on_update=[]
                    )
                    new_insts.append(nop.ins)
                inserts[inst.name] = new_insts
    if not inserts:
        return
    # nop() appended the new instructions to whatever bb was current; strip
    # them from everywhere, then re-insert each right before its owner so
    # the engine observes every wait before executing the instruction.
    appended = {ni.name for nis in inserts.values() for ni in nis}
    for f in nc.m.functions:
        for bb in f.blocks:
            rebuilt = []
            changed = False
            for inst in bb.instructions:
                if inst.name in appended:
                    changed = True
                    continue
                if inst.name in inserts:
                    rebuilt.extend(inserts[inst.name])
                    changed = True
                rebuilt.append(inst)
            if changed:
                bb.instructions = rebuilt


def _build_program():
    import concourse.bass as bass
    import concourse.mybir as mybir
    import concourse.tile as tile

    f32 = mybir.dt.float32
    f16 = mybir.dt.float16
    u32 = mybir.dt.uint32
    E = NUM_EXPERTS
    N_PASS = len(PASS_TOK)

    nc = bass.Bass("TRN2", target_bir_lowering=False, debug=False)

    # per-pass x blocks, host-reordered to [chunk][row in chunk][hi|lo][tok]
    # fp16: fully sequential DRAM, contiguous per-partition lines.
    xp_d = [
        nc.dram_tensor(f"xp{p}", [N_CHUNK, 128, 2, PASS_TOK[p]], f16,
                       kind="ExternalInput")
        for p in range(N_PASS)
    ]
    # proto hi|lo, host-reordered to [row in chunk][chunk][2E]
    phpl_d = nc.dram_tensor("phpl", [128, N_CHUNK, 2 * E], f16,
                            kind="ExternalInput")
    # [-gate on expert rows 0..63, 0 on rows 64..127]: the per-partition
    # bias for the fused a-staging op (see epilogue)
    gate2 = nc.dram_tensor("gate2", [128, 1], f32, kind="ExternalInput")
    w_out = nc.dram_tensor("w_out", [128, N_SUB * TOP_K], f32, kind="ExternalOutput")
    i_out = nc.dram_tensor("i_out", [128, N_SUB * TOP_K], u32, kind="ExternalOutput")

    ident_dram = nc.inline_tensor(np.eye(E, dtype=np.float32), name="ident64")

    with tile.TileContext(nc) as tc:
        with (
            tc.tile_pool(name="const", bufs=1) as const_pool,
            tc.tile_pool(name="xa", bufs=12) as x_pool,
            tc.tile_pool(name="acc", bufs=6, space="PSUM") as acc_pool,
            tc.tile_pool(name="tp", bufs=2, space="PSUM") as tp_pool,
            tc.tile_pool(name="lg", bufs=3) as lg_pool,
            tc.tile_pool(name="outp", bufs=1) as out_pool,
        ):
            # --- constants (gpsimd SWDGE ring, ahead of its x share) ---
            phpl_sb = const_pool.tile([128, N_CHUNK, 2 * E], f16)
            # chunk 0 separately so the first matmul's weights land early
            nc.gpsimd.dma_start(phpl_sb[:, 0, :], phpl_d[:, 0, :])
            nc.gpsimd.dma_start(phpl_sb[:, 1:, :], phpl_d[:, 1:, :])
            gate_sb = const_pool.tile([128, 1], f32)
            nc.gpsimd.dma_start(gate_sb[:], gate2[:])
            ident_sb = const_pool.tile([E, E], f32)
            nc.gpsimd.dma_start(ident_sb[:], ident_dram[:])

            vals_sb = out_pool.tile([128, N_SUB * TOP_K], f32)
            idx_sb = out_pool.tile([128, N_SUB * TOP_K], u32)

            # x streams on the two HWDGE rings.  With compute running, the
            # SBUF write ports shared with the PE's ~600 GB/s of operand
            # reads cap DMA near ~400 GB/s, which two rings already reach;
            # adding the SWDGE ring as a third x path measured SLOWER
            # (Q7 emission + descriptor-ring SBUF traffic).
            rings = [nc.sync, nc.scalar]
            ring_state = [0]

            def next_ring():
                r = rings[ring_state[0] % 2]
                ring_state[0] += 1
                return r

            def emit_epilogue(p, groups, goff, a_accs, b_accs, base):
                # tk = [a_hi + 2^-11 (a_lo + b_hi) + 2^-22 b_lo]/64 - gate,
                # per 128-token subtile, transposed to [token, expert].
                # ENTIRELY on the DVE (copies, recombines, scale+gate via
                # per-partition tensor_scalar, 32x32-block transpose,
                # Max8/MaxIndex8): the in-order PE queue stays pure
                # accumulation matmuls and the sync/scalar queues stay pure
                # DMA triggers, so the epilogue can never stall the x
                # stream.  Max8 runs pre-relu (relu is monotone, so top-8
                # order is unchanged); the relu clamp lands on the [*, 8]
                # outputs at pass end.
                gorder = (reversed(range(len(groups))) if p == N_PASS - 1
                          else range(len(groups)))
                gorder = list(gorder)
                # all fused `a` stagings first: a2 = a * 2^-6 + [-gate; 0]
                # (PSUM -> SBUF).  They free the PSUM banks the NEXT pass's
                # b-matmuls are waiting to reuse — if they queued behind a
                # group's full DVE chain, the PE would stall >3.4us at the
                # pass boundary and the hardware activity monitor would
                # downclock it to 1.2 GHz.  Folding the 1/sqrt(4096) scale
                # and gate bias here (exact power-of-2 rescale) also drops
                # one stage from the recombine chain.
                a_sbs = {}
                for g in gorder:
                    a_sb = lg_pool.tile(
                        [128, groups[g]], f32, name="a_sb", tag="a", bufs=2
                    )
                    nc.vector.tensor_scalar(
                        a_sb[:], a_accs[g][:], 1.0 / 64.0, gate_sb[:],
                        bass.mybir.AluOpType.mult, bass.mybir.AluOpType.add,
                    )
                    a_sbs[g] = a_sb
                for g in gorder:
                    W = groups[g]
                    nsub = W // 128
                    a_sb = a_sbs[g]
                    # sg = a2_hi + 2^-11 (u + 2^-6 b_lo... ):
                    #   u  = 2^-6 b_hi + a2_lo          = 2^-6 (a_lo + b_hi)
                    #   v  = 2^-17 b_lo + u
                    #   sg = 2^-11 v + a2_hi
                    #      = [a_hi + 2^-11 (a_lo+b_hi) + 2^-22 b_lo]/64 - gate
                    u = lg_pool.tile([E, W], f32, name="u", tag="u")
                    nc.vector.scalar_tensor_tensor(
                        u[:], b_accs[g][0:E, :], 1.0 / 64.0, a_sb[E:2 * E, :],
                        bass.mybir.AluOpType.mult, bass.mybir.AluOpType.add,
                    )
                    v = lg_pool.tile([E, W], f32, name="v", tag="v")
                    nc.vector.scalar_tensor_tensor(
                        v[:], b_accs[g][E:2 * E, :], 2.0 ** -17, u[:],
                        bass.mybir.AluOpType.mult, bass.mybir.AluOpType.add,
                    )
                    sg = lg_pool.tile([E, W], f32, name="sg", tag="s")
                    nc.vector.scalar_tensor_tensor(
                        sg[:], v[:], LO_UNSCALE, a_sb[0:E, :],
                        bass.mybir.AluOpType.mult, bass.mybir.AluOpType.add,
                    )
                    tk_psum = tp_pool.tile([128, 4 * E], f32, name="tk_psum")
                    for jj in range(nsub):
                        nc.tensor.transpose(
                            tk_psum[:, jj * E:(jj + 1) * E],
                            sg[:, jj * 128:(jj + 1) * 128],
                            ident_sb[:],
                        )
                    tk_sb = lg_pool.tile([128, 4 * E], f32, name="tk_sb", tag="t")
                    nc.vector.tensor_copy(
                        tk_sb[:, 0:nsub * E], tk_psum[:, 0:nsub * E]
                    )
                    gsub = base + goff[g] // 128
                    for jj in range(nsub):
                        s = gsub + jj
                        nc.vector.max(
                            vals_sb[:, s * TOP_K:(s + 1) * TOP_K],
                            tk_sb[:, jj * E:(jj + 1) * E],
                        )
                        nc.vector.max_index(
                            idx_sb[:, s * TOP_K:(s + 1) * TOP_K],
                            vals_sb[:, s * TOP_K:(s + 1) * TOP_K],
                            tk_sb[:, jj * E:(jj + 1) * E],
                        )
                # relu: clamp this pass's top-8 values (order-preserving)
                nsub_p = sum(groups) // 128
                os_ = slice(base * TOP_K, (base + nsub_p) * TOP_K)
                nc.vector.tensor_scalar_max(
                    vals_sb[:, os_], vals_sb[:, os_], 0.0
                )
                if p == N_PASS - 1:
                    # single output flush at the very end: every DMA ring
                    # carries x-stream triggers mid-run, and a flush trigger
                    # waiting on the epilogue would stall them (in-order
                    # queues).  By now both HWDGE rings are idle.
                    nc.sync.dma_start(w_out[:], vals_sb[:])
                    nc.scalar.dma_start(i_out[:], idx_sb[:])

            sub_base = 0  # running 128-token output subtile index
            pending = None  # deferred epilogue of the previous pass: its PE
            # matmuls must be emitted AFTER the next pass's accumulation
            # matmuls, or the in-order PE queue would stall the next pass's
            # compute (and therefore the x stream) on the epilogue's DVE
            # dependency chain.
            for p in range(N_PASS):
                T = PASS_TOK[p]
                groups = PASS_GROUPS[p]
                goff = np.cumsum([0] + groups)[:-1]
                a_accs = [
                    acc_pool.tile([128, w], f32, name=f"a_p{p}g{g}", tag="acc")
                    for g, w in enumerate(groups)
                ]
                b_accs = [
                    acc_pool.tile([128, w], f32, name=f"b_p{p}g{g}", tag="acc")
                    for g, w in enumerate(groups)
                ]

                # ---- x stream for this pass ----
                c = 0
                slot_of = {}
                last = N_PASS - 1
                while c < N_CHUNK:
                    if p == 0 and c == 0:
                        # split the very first chunk by stream and half so
                        # the first matmul waits on ~128 KB, not 1.5 MB
                        x_t = x_pool.tile([128, 1, 2, T], f16, name="x_h", tag="xt")
                        src = xp_d[p][0:1].rearrange("c r s t -> r c s t")
                        h = groups[0]
                        nc.sync.dma_start(x_t[:, 0, 0, 0:h], src[:, 0, 0, 0:h])
                        nc.scalar.dma_start(x_t[:, 0, 1, 0:h], src[:, 0, 1, 0:h])
                        nc.sync.dma_start(x_t[:, 0, 0, h:], src[:, 0, 0, h:])
                        nc.scalar.dma_start(x_t[:, 0, 1, h:], src[:, 0, 1, h:])
                        slot_of[0] = (x_t, 0)
                        c += 1
                    elif p == last and c == N_CHUNK - 2:
                        # final two chunks: one small transfer per HWDGE ring
                        # (lowest latency) so the last bytes land soon
                        x_t = x_pool.tile([128, 2, 2, T], f16, name="x_z", tag="xt")
                        src = xp_d[p][c:c + 2].rearrange("c r s t -> r c s t")
                        nc.sync.dma_start(x_t[:, 0], src[:, 0])
                        nc.scalar.dma_start(x_t[:, 1], src[:, 1])
                        slot_of[c] = (x_t, 0)
                        slot_of[c + 1] = (x_t, 1)
                        c += 2
                    else:
                        n = min(PASS_CHDMA[p], N_CHUNK - c)
                        if p == 0 and c <= 4:
                            n = 1  # small first transfers: fast PE ramp
                        if p == last:
                            n = min(n, N_CHUNK - 2 - c)
                        x_t = x_pool.tile([128, n, 2, T], f16, name="x_t", tag="xt")
                        src = xp_d[p][c:c + n].rearrange("c r s t -> r c s t")
                        next_ring().dma_start(x_t[:], src)
                        for j in range(n):
                            slot_of[c + j] = (x_t, j)
                        c += n

                # ---- accumulation matmuls ----
                # The previous pass's epilogue (which contains PE matmuls
                # gated on its DVE recombines) is emitted a few chunks INTO
                # this pass's matmul stream: late enough that its DVE inputs
                # are ready when the in-order PE reaches it, early enough
                # that its downstream DVE work isn't stuck behind this whole
                # pass (the PE absorbs the extra matmuls with its slack).
                for c in range(N_CHUNK):
                    x_t, j = slot_of[c]
                    first, lastc = (c == 0), (c == N_CHUNK - 1)
                    grange = (
                        reversed(range(len(groups)))
                        if (lastc and p == N_PASS - 1)
                        else range(len(groups))
                    )
                    for g in grange:
                        ts = slice(goff[g], goff[g] + groups[g])
                        nc.tensor.matmul(
                            a_accs[g][:], phpl_sb[:, c, :], x_t[:, j, 0, ts],
                            start=first, stop=lastc,
                        )
                        nc.tensor.matmul(
                            b_accs[g][:], phpl_sb[:, c, :], x_t[:, j, 1, ts],
                            start=first, stop=lastc,
                        )
                    if c == 6 and pending is not None:
                        emit_epilogue(*pending)
                        pending = None

                pending = (p, groups, goff, a_accs, b_accs, sub_base)
                sub_base += T // 128
            emit_epilogue(*pending)

    _split_multi_waits(nc)
    return nc


def _get_program():
    global _PROGRAM
    if _PROGRAM is None:
        _PROGRAM = _build_program()
    return _PROGRAM


def _make_in_maps(x, proto_k, gate):
    xf = np.ascontiguousarray(x, dtype=np.float32).reshape(TOKENS, HIDDEN)
    proto = np.asarray(proto_k, dtype=np.float32)
    ph = proto.astype(np.float16)
    pl = ((proto - ph.astype(np.float32)) * LO_SCALE).astype(np.float16)
    phpl = np.concatenate([ph.T, pl.T], axis=1)           # [4096, 128] f16
    # [row in chunk][chunk][2E]
    phpl_r = np.ascontiguousarray(
        phpl.reshape(N_CHUNK, 128, 2 * NUM_EXPERTS).transpose(1, 0, 2)
    )
    gate2 = np.zeros((128, 1), np.float32)
    gate2[:NUM_EXPERTS, 0] = -np.asarray(gate, dtype=np.float32)
    toff = np.cumsum([0] + PASS_TOK)
    in_maps = []
    for cid in range(N_CORES):
        shard = xf[cid * T_CORE:(cid + 1) * T_CORE]       # [2048, 4096]
        hi = shard.astype(np.float16)
        lo = ((shard - hi.astype(np.float32)) * LO_SCALE).astype(np.float16)
        hi_t = hi.T.reshape(N_CHUNK, 128, T_CORE)         # [chunk, row, tok]
        lo_t = lo.T.reshape(N_CHUNK, 128, T_CORE)
        m = {"phpl": phpl_r, "gate2": gate2}
        for p, T in enumerate(PASS_TOK):
            xp = np.empty((N_CHUNK, 128, 2, T), np.float16)
            xp[:, :, 0, :] = hi_t[:, :, toff[p]:toff[p + 1]]
            xp[:, :, 1, :] = lo_t[:, :, toff[p]:toff[p + 1]]
            m[f"xp{p}"] = xp
        in_maps.append(m)
    return in_maps


def _gather(results):
    w = np.empty((TOKENS, TOP_K), np.float32)
    idx = np.empty((TOKENS, TOP_K), np.int32)
    for c in range(N_CORES):
        wo = results[c]["w_out"]                          # [128, 16*8]
        io = results[c]["i_out"].view(np.int32)
        w[c * T_CORE:(c + 1) * T_CORE] = (
            wo.reshape(128, N_SUB, TOP_K).transpose(1, 0, 2).reshape(T_CORE, TOP_K)
        )
        idx[c * T_CORE:(c + 1) * T_CORE] = (
            io.reshape(128, N_SUB, TOP_K).transpose(1, 0, 2).reshape(T_CORE, TOP_K)
        )
    return w.reshape(4, 4096, TOP_K), idx.reshape(4, 4096, TOP_K)


def run_sharded(in_maps, trace=False, trace_cores=None):
    from concourse.bass_utils import run_bass_kernel_spmd

    nc = _get_program()
    return run_bass_kernel_spmd(
        nc,
        in_maps,
        core_ids=list(range(N_CORES)),
        trace=trace,
        trace_cores=trace_cores,
    )


def kernel(x, proto_k, gate):
    in_maps = _make_in_maps(x, proto_k, gate)
    res = run_sharded(in_maps, trace=False)
    return _gather(res.results)
